# revision 1
# baseline (speedup 1.0000x reference)
"""2-layer GAT (nn_GAT_31490700214331) on 8 Trainium2 NeuronCores.

Strategy (dst-sharded, SPMD, per-core-rotated node layout):
  - Nodes are block-partitioned: core c owns nodes [c*6250, (c+1)*6250).
  - Every table on core c uses a ROTATED row order: node n lives at row
    (n - c*6250) mod 50000, so each core's own nodes are rows 0..6249 and
    the single SPMD program has no core-dependent offsets — the rotation
    lives entirely in host-prepared index/input arrays.
  - Layer-0 features (h0 = x @ W0) + attention alphas are computed
    replicated on every core (cheap) into a rotated DRAM table; edges are
    grouped by dst tile (128 dsts) and their source rows fetched with
    dma_gather (int16 indices -> the table is gathered through two views,
    rows [0, SPLIT) and [SPLIT, ...), keeping indices < 32768).
  - Edge softmax (safe without segment-max: |e| <= ~5) and the weighted
    aggregation are fused into per-chunk 128x128 incidence matmuls
    accumulating in PSUM; denominators ride along as 8 extra columns.
  - Between layers the ELU'd hidden state is AllGather'd (feature-major),
    rotated into per-core order with partition-id-offset DMA copies, and
    layer 1 repeats the scheme with 512-wide features and a head-mean +
    log_softmax epilogue.
  - alpha projections fold into the weight matmuls on the host:
    h @ blockdiag(a) == x @ (W @ blockdiag(a)), so the device gets
    W0a=[256,16] / W1a=[128,16] and computes alphas as 16 extra psum cols.

Self-contained: call kernel(**inputs) with the full-problem arrays.
"""
import numpy as np
from contextlib import ExitStack

import concourse.bacc as bacc
import concourse.bass as bass
import concourse.mybir as mybir
from concourse.tile import TileContext
from concourse.bass_utils import run_bass_kernel_spmd

F16 = mybir.dt.float16
F32 = mybir.dt.float32
I16 = mybir.dt.int16

N = 50000
NFEAT = 256
NHID = 128
NCLASS = 64
HEADS = 8
SLOPE = 0.2
NCORES = 8
NLOC = N // NCORES           # 6250
LT = (NLOC + 127) // 128     # 49 local dst tiles
LAST_ROWS = NLOC - (LT - 1) * 128   # 106 rows in the last tile
GT = 392                     # global node tiles (392*128 = 50176)
GROWS = GT * 128
SPLIT = 25088                # low/high gather-table split (196 tiles)
SENT = 300.0                 # dst_rel sentinel for padding slots
T0W = 256                    # t0 row: [h0(128)|as0(8)|ad0(8)|junk] f16
T1W = 640                    # t1 row: [h1(512)|as1(8)|ad1(8)|junk] f16

_cache = {}


# --------------------------------------------------------------------------
# host-side preparation
# --------------------------------------------------------------------------

def _wrap_idx(idx):
    """[n] int -> [128, n//16] int16 wrapped gather-index layout."""
    n = idx.shape[0]
    assert n % 16 == 0
    w = idx.reshape(n // 16, 16).T.astype(np.int16)
    return np.tile(w, (8, 1))


def _prep_edges(src, dst):
    cores = []
    for c in range(NCORES):
        m = (dst >= c * NLOC) & (dst < (c + 1) * NLOC)
        s = src[m].astype(np.int64)
        d = dst[m].astype(np.int64) - c * NLOC
        order = np.argsort(d, kind="stable")
        s, d = s[order], d[order]
        s_rot = (s - c * NLOC) % N
        tiles = []
        for t in range(LT):
            sel = (d >= t * 128) & (d < (t + 1) * 128)
            st, dt = s_rot[sel], d[sel] - t * 128
            lo = st < SPLIT
            tiles.append((st[lo], dt[lo], st[~lo] - SPLIT, dt[~lo]))
        cores.append(tiles)
    nl = max(len(t[0]) for tl in cores for t in tl)
    nh = max(len(t[2]) for tl in cores for t in tl)
    NL = max(1, (nl + 127) // 128)
    NH = max(1, (nh + 127) // 128)
    assert NL * 128 <= 1024 and NH * 128 <= 1024, (NL, NH)

    out = []
    for c in range(NCORES):
        eil = np.zeros((LT, 128, NL * 8), np.int16)
        eih = np.zeros((LT, 128, NH * 8), np.int16)
        eal = np.zeros((LT, 128, NL * 8), np.int16)
        eah = np.zeros((LT, 128, NH * 8), np.int16)
        drel = np.full((LT, 128, NL + NH), SENT, np.float16)
        for t in range(LT):
            sl, dl, sh, dh = cores[c][t]
            il = np.zeros(NL * 128, np.int64)
            il[: len(sl)] = sl
            al = np.zeros(NL * 128, np.int64)
            al[: len(dl)] = t * 128 + dl
            ih = np.zeros(NH * 128, np.int64)
            ih[: len(sh)] = sh
            ah = np.zeros(NH * 128, np.int64)
            ah[: len(dh)] = t * 128 + dh
            eil[t] = _wrap_idx(il)
            eih[t] = _wrap_idx(ih)
            eal[t] = _wrap_idx(al)
            eah[t] = _wrap_idx(ah)
            rl = np.full(NL * 128, SENT)
            rl[: len(dl)] = dl
            rh = np.full(NH * 128, SENT)
            rh[: len(dh)] = dh
            r = np.concatenate([rl, rh]).reshape(NL + NH, 128).T
            drel[t] = r.astype(np.float16)
        epack = np.concatenate(
            [eil, eih, eal, eah, drel.view(np.int16)], axis=2)
        out.append(dict(epack=np.ascontiguousarray(epack)))
    return NL, NH, out


def _prep_inputs(x, edge_index, W0, a_src0, a_dst0, b0, W1, a_src1, a_dst1,
                 b1):
    src = np.asarray(edge_index[0]).astype(np.int64)
    dst = np.asarray(edge_index[1]).astype(np.int64)
    NL, NH, edata = _prep_edges(src, dst)

    def bd(a):  # [H, D] -> blockdiag [H*D, H]
        a = np.asarray(a, np.float32)
        H, D = a.shape
        m = np.zeros((H * D, H), np.float32)
        for h in range(H):
            m[h * D:(h + 1) * D, h] = a[h]
        return m

    W0 = np.asarray(W0, np.float32)
    W1 = np.asarray(W1, np.float32)
    W0a = np.concatenate([W0 @ bd(a_src0), W0 @ bd(a_dst0)], 1)  # [256, 16]
    # head-innermost feature interleave: new col d*8+h <- old col h*D+d
    perm0 = np.array([(f % 8) * 16 + f // 8 for f in range(128)])
    perm1 = np.array([(f % 8) * 64 + f // 8 for f in range(512)])
    W0cat = np.concatenate([W0[:, perm0], W0a], 1)               # [256, 144]
    W1a = np.concatenate([W1 @ bd(a_src1), W1 @ bd(a_dst1)], 1)  # [128, 16]

    x = np.asarray(x, np.float32)
    ident = np.eye(128, dtype=np.float16)
    colio = np.tile(np.arange(128, dtype=np.float16)[None, :], (128, 1))
    b0b = np.tile(np.asarray(b0, np.float32)[None, :], (128, 1))
    b1b = np.tile(np.asarray(b1, np.float32)[None, :], (128, 1))

    in_maps = []
    for c in range(NCORES):
        rot = np.roll(np.arange(N), -c * NLOC)
        xr = np.zeros((GROWS, NFEAT), np.float16)
        xr[:N] = x[rot].astype(np.float16)
        xtt = xr.reshape(GROWS // 128, 128, 2, 128).transpose(0, 3, 2, 1)
        m = dict(
            xT=np.ascontiguousarray(xtt),
            W0=np.ascontiguousarray(
                W0cat.astype(np.float16).reshape(2, 128, NHID + 16)),
            W1=np.ascontiguousarray(W1[perm0][:, perm1].astype(np.float16)),
            W1a=np.ascontiguousarray(W1a[perm0].astype(np.float16)),
            b0b=np.ascontiguousarray(b0b[:, perm0]), b1b=b1b,
            ident=ident, colio=colio,
            **edata[c],
        )
        in_maps.append(m)
    return NL, NH, in_maps


# --------------------------------------------------------------------------
# device program
# --------------------------------------------------------------------------

def build(NL, NH, lt=LT, gt=GT, debug=False, phases="ABCDE"):
    CH = NL + NH
    HID16 = NHID + 16
    NLI = NL * 128
    NHI = NH * 128

    EPW = NL * 8 + NH * 8 + NL * 8 + NH * 8 + CH   # packed int16 cols
    nc = bacc.Bacc("TRN2")
    xT = nc.dram_tensor("xT", [GROWS // 128, 128, 2, 128], F16,
                        kind="ExternalInput")
    W0i = nc.dram_tensor("W0", [2, 128, NHID + 16], F16,
                         kind="ExternalInput")
    W1i = nc.dram_tensor("W1", [NHID, 512], F16, kind="ExternalInput")
    W1ai = nc.dram_tensor("W1a", [NHID, 16], F16, kind="ExternalInput")
    b0bi = nc.dram_tensor("b0b", [128, NHID], F32, kind="ExternalInput")
    b1bi = nc.dram_tensor("b1b", [128, NCLASS], F32, kind="ExternalInput")
    identi = nc.dram_tensor("ident", [128, 128], F16, kind="ExternalInput")
    colioi = nc.dram_tensor("colio", [128, 128], F16, kind="ExternalInput")
    epacki = nc.dram_tensor("epack", [lt, 128, EPW], I16,
                            kind="ExternalInput")
    out = nc.dram_tensor("out", [NLOC, NCLASS], F32, kind="ExternalOutput")
    dbg = None
    if debug:
        dbg = nc.dram_tensor("dbg", [gt * 128, T0W], F32,
                             kind="ExternalOutput")

    with TileContext(nc) as tc, ExitStack() as stk:
        reg_l = nc.gpsimd.to_reg(NLI)
        reg_h = nc.gpsimd.to_reg(NHI)
        dpool = stk.enter_context(
            tc.tile_pool(name="dram", bufs=1, space="DRAM"))
        t0lo = dpool.tile([SPLIT, T0W], F16, tag="t0lo")
        t0hi = dpool.tile([GROWS - SPLIT, T0W], F16, tag="t0hi")
        t0ad = dpool.tile([lt * 128, 128], F16, tag="t0ad")
        t1lo = dpool.tile([SPLIT, T1W], F16, tag="t1lo")
        t1hi = dpool.tile([GROWS - SPLIT, T1W], F16, tag="t1hi")
        t1ad = dpool.tile([lt * 128, 128], F16, tag="t1ad")
        agin = dpool.tile([128, NLOC], F16, tag="agin")
        agout = dpool.tile([NCORES * 128, NLOC], F16, tag="agout",
                           addr_space="Shared")

        cpool = stk.enter_context(tc.tile_pool(name="const", bufs=1))
        W0s = cpool.tile([128, 2, NHID + 16], F16)
        nc.sync.dma_start(out=W0s[:], in_=W0i.rearrange("k p n -> p k n"))
        W1s = cpool.tile([128, 512], F16)
        nc.sync.dma_start(out=W1s[:], in_=W1i[:])
        W1as = cpool.tile([128, 16], F16)
        nc.sync.dma_start(out=W1as[:], in_=W1ai[:])
        b0s = cpool.tile([128, NHID], F32)
        nc.sync.dma_start(out=b0s[:], in_=b0bi[:])
        b1s = cpool.tile([128, NCLASS], F32)
        nc.sync.dma_start(out=b1s[:], in_=b1bi[:])
        idents = cpool.tile([128, 128], F16)
        nc.sync.dma_start(out=idents[:], in_=identi[:])
        colios = cpool.tile([128, 128], F16)
        nc.sync.dma_start(out=colios[:], in_=colioi[:])
        zeros = cpool.tile([128, 128], F16)
        nc.vector.memset(zeros[:], 0)

        # ---------------- phase A: layer-0 tables (replicated) ------------
        with ExitStack() as pa:
            xp = pa.enter_context(tc.tile_pool(name="pa_x", bufs=4))
            pp = pa.enter_context(
                tc.tile_pool(name="pa_ps", bufs=2, space="PSUM"))
            rp = pa.enter_context(tc.tile_pool(name="pa_row", bufs=4))
            assert gt % 2 == 0
            for gg in range(gt // 2):
                xa = xp.tile([128, 2, 2, 128], F16, tag="xa")
                nc.sync.dma_start(
                    out=xa[:],
                    in_=xT[2 * gg:2 * gg + 2].rearrange(
                        "g p k j -> p g k j"))
                row = rp.tile([128, 2, T0W], F16, tag="row")
                pss = []
                for g2 in range(2):
                    ps = pp.tile([128, HID16], F32, tag=f"ps{g2}")
                    for k in range(2):
                        nc.tensor.matmul(ps[:], xa[:, g2, k, :],
                                         W0s[:, k, :],
                                         start=(k == 0), stop=(k == 1))
                    nc.scalar.copy(row[:, g2, 0:HID16], ps[:])
                    pss.append(ps)
                eng = nc.scalar if gg % 2 else nc.sync
                g0 = 2 * gg * 128
                if g0 + 256 <= SPLIT:
                    eng.dma_start(
                        out=t0lo[g0:g0 + 256, :]
                        .rearrange("(g p) w -> p g w", p=128),
                        in_=row[:])
                else:
                    o = g0 - SPLIT
                    eng.dma_start(
                        out=t0hi[o:o + 256, :]
                        .rearrange("(g p) w -> p g w", p=128),
                        in_=row[:])
                for g2 in range(2):
                    g = 2 * gg + g2
                    if g < lt:
                        adr = rp.tile([128, 8], F16, tag="adr")
                        nc.vector.tensor_copy(
                            adr[:], pss[g2][:, NHID + 8:HID16])
                        nc.sync.dma_start(
                            out=t0ad[g * 128:(g + 1) * 128, 0:8],
                            in_=adr[:])
                if debug:
                    rowd = rp.tile([128, T0W], F32, tag="rowd")
                    nc.vector.tensor_copy(rowd[:, 0:HID16], row[:, 0:HID16])
                    nc.vector.memset(rowd[:, HID16:T0W], 0)
                    nc.sync.dma_start(
                        out=dbg[g * 128:(g + 1) * 128, :], in_=rowd[:])

        # ---------------- shared edge phase -------------------------------
        def edge_phase(layer, tbl_lo, tbl_hi, tblad, fdim, trow, rw,
                       post_fn, fin):
            o_il, o_ih = 0, NL * 8
            o_al, o_ah = NL * 16, NL * 16 + NH * 8
            o_dr = NL * 16 + NH * 16
            with ExitStack() as pb:
                ip = pb.enter_context(
                    tc.tile_pool(name=f"ix{layer}", bufs=4))
                gp = pb.enter_context(
                    tc.tile_pool(name=f"gg{layer}", bufs=4))
                apl = pb.enter_context(
                    tc.tile_pool(name=f"ga{layer}", bufs=3))
                rp2 = pb.enter_context(
                    tc.tile_pool(name=f"rh{layer}", bufs=3))
                pp2 = pb.enter_context(
                    tc.tile_pool(name=f"ps{layer}", bufs=2, space="PSUM"))
                op = pb.enter_context(
                    tc.tile_pool(name=f"po{layer}", bufs=3))
                for t in range(lt):
                    ep = ip.tile([128, EPW], I16, tag="ep")
                    nc.sync.dma_start(out=ep[:], in_=epacki[t])
                    il = ep[:, o_il:o_il + NL * 8]
                    ih = ep[:, o_ih:o_ih + NH * 8]
                    al = ep[:, o_al:o_al + NL * 8]
                    ah = ep[:, o_ah:o_ah + NH * 8]
                    dr = ep[:, o_dr:o_dr + CH].bitcast(F16)

                    G = gp.tile([128, CH, trow], F16, tag="G")
                    nc.gpsimd.dma_gather(G[:, 0:NL, :], tbl_lo[:], il,
                                         NLI, reg_l, trow)
                    nc.gpsimd.dma_gather(G[:, NL:CH, :], tbl_hi[:],
                                         ih, NHI, reg_h, trow)
                    A = apl.tile([128, CH, 128], F16, tag="A")
                    nc.gpsimd.dma_gather(A[:, 0:NL, :], tblad[:], al,
                                         NLI, reg_l, 128)
                    nc.gpsimd.dma_gather(A[:, NL:CH, :], tblad[:], ah,
                                         NHI, reg_h, 128)

                    inc = rp2.tile([128, CH, 128], F16, tag="inc")
                    nc.vector.tensor_tensor(
                        out=inc[:],
                        in0=dr.unsqueeze(-1).broadcast_to([128, CH, 128]),
                        in1=colios[:].unsqueeze(1)
                        .broadcast_to([128, CH, 128]),
                        op=mybir.AluOpType.is_equal)
                    EX = rp2.tile([128, CH, 8], F16, tag="EX")
                    nc.vector.tensor_tensor(
                        out=EX[:], in0=G[:, :, fdim:fdim + 8],
                        in1=A[:, :, 0:8], op=mybir.AluOpType.add)
                    nc.scalar.activation(
                        EX[:], EX[:], mybir.ActivationFunctionType.Prelu,
                        alpha=SLOPE)
                    nc.scalar.activation(
                        EX[:], EX[:], mybir.ActivationFunctionType.Exp)

                    R = rp2.tile([128, CH, fdim], F16, tag="R")
                    H = HEADS
                    D = fdim // H
                    nc.vector.tensor_tensor(
                        out=R[:, :, 0:fdim]
                        .rearrange("p c (d h) -> p c d h", h=H),
                        in0=G[:, :, 0:fdim]
                        .rearrange("p c (d h) -> p c d h", h=H),
                        in1=EX[:].unsqueeze(2).broadcast_to([128, CH, D, H]),
                        op=mybir.AluOpType.mult)

                    P1 = pp2.tile([128, fdim], F32, tag="P1")
                    P2 = pp2.tile([128, 8], F32, tag="P2")
                    for ch in range(CH):
                        nc.tensor.matmul(P1[:], inc[:, ch, :],
                                         R[:, ch, 0:fdim],
                                         start=(ch == 0),
                                         stop=(ch == CH - 1))
                    for ch in range(CH):
                        nc.tensor.matmul(P2[:], inc[:, ch, :],
                                         EX[:, ch, :],
                                         start=(ch == 0),
                                         stop=(ch == CH - 1))
                    post_fn(t, P1, P2, op, pp2, fin)

        # ---- L0 post: softmax-div, +b0, ELU, transpose, store ------------
        def post0(t, P1, P2, op, pp2, fin):
            rows = 128 if t < lt - 1 else LAST_ROWS
            r8 = op.tile([128, 8], F32, tag="r8")
            nc.vector.tensor_scalar_add(r8[:], P2[:], 1e-16)
            nc.vector.reciprocal(r8[:], r8[:])
            z = op.tile([128, NHID], F32, tag="z")
            nc.vector.tensor_tensor(
                out=z[:].rearrange("p (d h) -> p d h", h=HEADS),
                in0=P1[:].rearrange("p (d h) -> p d h", h=HEADS),
                in1=r8[:].unsqueeze(1).broadcast_to([128, 16, HEADS]),
                op=mybir.AluOpType.mult)
            nc.vector.tensor_tensor(out=z[:], in0=z[:], in1=b0s[:],
                                    op=mybir.AluOpType.add)
            zm = op.tile([128, NHID], F32, tag="zm")
            nc.vector.tensor_scalar_min(zm[:], z[:], 0.0)
            nc.scalar.activation(zm[:], zm[:],
                                 mybir.ActivationFunctionType.Exp)
            zp = op.tile([128, NHID], F32, tag="zp")
            nc.vector.tensor_scalar_max(zp[:], z[:], 0.0)
            nc.vector.tensor_tensor(out=zp[:], in0=zp[:], in1=zm[:],
                                    op=mybir.AluOpType.add)
            h1 = op.tile([128, NHID], F16, tag="h1")
            nc.vector.tensor_scalar_add(h1[:], zp[:], -1.0)
            pst = pp2.tile([128, 128], F16, tag="pst")
            nc.tensor.transpose(pst[:], h1[:], idents[:])
            hT = op.tile([128, 128], F16, tag="hT")
            nc.vector.tensor_copy(hT[:], pst[:])
            nc.sync.dma_start(
                out=agin[:, t * 128:t * 128 + rows], in_=hT[:, 0:rows])

        if "B" in phases:
            edge_phase(0, t0lo, t0hi, t0ad, NHID, T0W, 8 + NHID, post0, None)

        # ---------------- phase C: AllGather + rotation -------------------
        sregs = None
        if "C" in phases:
            nc.gpsimd.collective_compute(
                "AllGather", mybir.AluOpType.bypass,
                replica_groups=[list(range(NCORES))],
                ins=[agin[:]], outs=[agout[:]])
            pid = nc.partition_id(engines=[mybir.EngineType.SP])
            sregs = [nc.sync.snap(((j + pid) % NCORES) * 128)
                     for j in range(NCORES)]

        # ---------------- phase D: layer-1 tables -------------------------
        with ExitStack() as pd:
            if "D" not in phases:
                pd.enter_context(ExitStack())  # keep structure
            ngt = min(gt, (N + 127) // 128)
            dsup = [(a, min(a + 2, ngt)) for a in range(0, ngt, 2)]
            if "D" not in phases:
                dsup = []
            xp1 = pd.enter_context(tc.tile_pool(name="pd_x", bufs=4))
            pp1 = pd.enter_context(
                tc.tile_pool(name="pd_ps", bufs=2, space="PSUM"))
            rp1 = pd.enter_context(tc.tile_pool(name="pd_row", bufs=4))
            for ga, gb in dsup:
                nsub = gb - ga
                hx = xp1.tile([128, 2, 128], F16, tag="hx")
                r0, r1 = ga * 128, min(gb * 128, N)
                hxf = hx[:].rearrange("p g j -> p (g j)")
                w0 = 0
                r = r0
                while r < r1:
                    j = r // NLOC
                    seg = min(r1, (j + 1) * NLOC) - r
                    nc.sync.dma_start(
                        out=hxf[:, w0:w0 + seg],
                        in_=agout[bass.ds(sregs[j % NCORES], 128),
                                  r - j * NLOC:r - j * NLOC + seg])
                    w0 += seg
                    r += seg
                row = rp1.tile([128, 2, T1W], F16, tag="row")
                psas = []
                for g2 in range(nsub):
                    ps = pp1.tile([128, 512], F32, tag=f"ps{g2}")
                    nc.tensor.matmul(ps[:], hx[:, g2, :], W1s[:],
                                     start=True, stop=True)
                    psa = pp1.tile([128, 16], F32, tag=f"psa{g2}")
                    nc.tensor.matmul(psa[:], hx[:, g2, :], W1as[:],
                                     start=True, stop=True)
                    nc.scalar.copy(row[:, g2, 0:256], ps[:, 0:256])
                    nc.vector.tensor_copy(row[:, g2, 256:512],
                                          ps[:, 256:512])
                    nc.vector.tensor_copy(row[:, g2, 512:528], psa[:])
                    psas.append(psa)
                eng = nc.scalar if ga % 4 else nc.sync
                g0 = ga * 128
                if nsub == 2 and g0 + 256 <= SPLIT:
                    eng.dma_start(
                        out=t1lo[g0:g0 + 256, 0:528]
                        .rearrange("(g p) w -> p g w", p=128),
                        in_=row[:, :, 0:528])
                elif nsub == 2:
                    o = g0 - SPLIT
                    eng.dma_start(
                        out=t1hi[o:o + 256, 0:528]
                        .rearrange("(g p) w -> p g w", p=128),
                        in_=row[:, :, 0:528])
                else:
                    o = g0 - SPLIT
                    eng.dma_start(out=t1hi[o:o + 128, 0:528],
                                  in_=row[:, 0, 0:528])
                for g2 in range(nsub):
                    g = ga + g2
                    if g < lt:
                        adr = rp1.tile([128, 8], F16, tag="adr")
                        nc.vector.tensor_copy(adr[:], psas[g2][:, 8:16])
                        nc.sync.dma_start(
                            out=t1ad[g * 128:(g + 1) * 128, 0:8],
                            in_=adr[:])

        # ---------------- phase E: layer-1 edges + epilogue ---------------
        def post1(t, P1, P2, op, pp2, fin):
            zbig, nmxb, seb = fin
            r8 = op.tile([128, 8], F32, tag="r8")
            nc.vector.tensor_scalar_add(r8[:], P2[:], 1e-16)
            nc.vector.reciprocal(r8[:], r8[:])
            nc.vector.tensor_scalar_mul(r8[:], r8[:], 1.0 / HEADS)
            zw = op.tile([128, 512], F32, tag="zw")
            nc.vector.tensor_tensor(
                out=zw[:].rearrange("p (d h) -> p d h", h=HEADS),
                in0=P1[:].rearrange("p (d h) -> p d h", h=HEADS),
                in1=r8[:].unsqueeze(1).broadcast_to([128, 64, HEADS]),
                op=mybir.AluOpType.mult)
            z = zbig[:, t * NCLASS:(t + 1) * NCLASS]
            nc.vector.reduce_sum(
                z, zw[:].rearrange("p (d h) -> p d h", h=HEADS),
                axis=mybir.AxisListType.X)
            nc.vector.tensor_tensor(out=z, in0=z, in1=b1s[:],
                                    op=mybir.AluOpType.add)
            nmx = nmxb[:, t:t + 1]
            nc.vector.reduce_max(nmx, z, axis=mybir.AxisListType.X,
                                 negate=True)
            ez = op.tile([128, NCLASS], F32, tag="ez")
            nc.scalar.activation(ez[:], z,
                                 mybir.ActivationFunctionType.Exp,
                                 bias=nmx, accum_out=seb[:, t:t + 1])

        if "E" in phases:
            fpool = stk.enter_context(tc.tile_pool(name="fin", bufs=1))
            zbig = fpool.tile([128, lt * NCLASS], F32)
            nmxb = fpool.tile([128, lt], F32)
            seb = fpool.tile([128, lt], F32)
            edge_phase(1, t1lo, t1hi, t1ad, 512, T1W, 520, post1,
                       (zbig, nmxb, seb))
            # batched log-softmax tail: one Ln + two broadcast ops + 2 DMAs
            nc.scalar.activation(seb[:], seb[:],
                                 mybir.ActivationFunctionType.Ln)
            nc.vector.tensor_tensor(
                out=zbig[:].rearrange("p (t c) -> p t c", c=NCLASS),
                in0=zbig[:].rearrange("p (t c) -> p t c", c=NCLASS),
                in1=nmxb[:].unsqueeze(-1).broadcast_to([128, lt, NCLASS]),
                op=mybir.AluOpType.add)
            nc.vector.tensor_tensor(
                out=zbig[:].rearrange("p (t c) -> p t c", c=NCLASS),
                in0=zbig[:].rearrange("p (t c) -> p t c", c=NCLASS),
                in1=seb[:].unsqueeze(-1).broadcast_to([128, lt, NCLASS]),
                op=mybir.AluOpType.subtract)
            nfull = (lt - 1) * 128
            rlast = LAST_ROWS if lt == LT else 128
            nc.sync.dma_start(
                out=out[0:nfull, :].rearrange("(t p) c -> p t c", p=128),
                in_=zbig[:].rearrange("p (t c) -> p t c", c=NCLASS)
                [:, 0:lt - 1, :])
            nc.sync.dma_start(
                out=out[nfull:nfull + rlast, :],
                in_=zbig[0:rlast, (lt - 1) * NCLASS:lt * NCLASS])

    nc.compile()
    return nc


# --------------------------------------------------------------------------
# entry point
# --------------------------------------------------------------------------

def kernel(**inputs) -> np.ndarray:
    NLk, NHk, in_maps = _prep_inputs(**inputs)
    key = (NLk, NHk)
    if key not in _cache:
        _cache[key] = build(NLk, NHk)
    nc = _cache[key]
    res = run_bass_kernel_spmd(nc, in_maps, list(range(NCORES)))
    return np.concatenate([res.results[c]["out"] for c in range(NCORES)], 0)



# revision 7
# speedup vs baseline: 1.1160x; 1.1160x over previous
"""2-layer GAT (nn_GAT_31490700214331) on 8 Trainium2 NeuronCores.

Strategy (dst-sharded, SPMD, per-core-rotated node layout) — v2:
  - Nodes block-partitioned: core c owns nodes [c*6250, (c+1)*6250); every
    table on core c uses a ROTATED row order (node n at row (n - c*6250)
    mod 50000) so one SPMD program serves all cores.
  - Phase A (replicated): h0 = x @ [W0 | W0·a_src | W0·a_dst] for all
    nodes; rows [h0|as0] land in the gather tables t0lo/t0hi (512 B rows),
    dst-alphas accumulate in SBUF and are written to t0ad in one DMA.
  - Phase B: per group of 4 dst tiles, three batched dma_gathers (src rows
    lo/hi + per-edge dst-alpha); edge softmax (safe without segment-max)
    and aggregation run as 128x128 incidence matmuls; denominators ride
    as 8 fused psum columns.  Incidence builds alternate DVE/GpSimd.
  - The hidden state is ELU'd, transposed, quantized to f8e4 and
    AllGather'd in 4 column chunks that overlap phase B's tail and
    phase D's head (COLLECTIVE_CORES runs concurrently with compute).
  - Phase D: supertiles of 8 node tiles, ordered by which AllGather chunk
    they need (own-core rows first, straight from local agin); the f8
    hidden state feeds mixed-precision matmuls with W1/W1a; rows
    [h1|as1] go to t1lo/t1hi (1280 B rows), dst-alphas to t1ad.
  - Phase E: like B with 640-col rows, separate denominator chain, and a
    head-mean + batched log_softmax epilogue.
  - alpha projections fold into the weight matmuls on the host:
    h @ blockdiag(a) == x @ (W @ blockdiag(a)).

Self-contained: call kernel(**inputs) with the full-problem arrays.
"""
import numpy as np
from contextlib import ExitStack

import concourse.bacc as bacc
import concourse.bass as bass
import concourse.mybir as mybir
from concourse.tile import TileContext
from concourse.bass_utils import run_bass_kernel_spmd

F16 = mybir.dt.float16
F32 = mybir.dt.float32
F8 = mybir.dt.float8e4
I16 = mybir.dt.int16
I8 = mybir.dt.int8

N = 50000
NFEAT = 256
NHID = 128
NCLASS = 64
HEADS = 8
SLOPE = 0.2
NCORES = 8
NLOC = N // NCORES           # 6250
LT = (NLOC + 127) // 128     # 49 local dst tiles
LAST_ROWS = NLOC - (LT - 1) * 128   # 106 rows in the last tile
GT = 392                     # global node tiles (392*128 = 50176)
GROWS = GT * 128
SPLIT = 25088                # low/high gather-table split (196 tiles)
SENT = 300.0                 # dst_rel sentinel for padding slots
T0W = 256                    # t0 row: [h0(128)|as0(8)|junk] f16
T1W = 640                    # t1 row: [h1(512)|as1(8)|junk] f16
STB = 4                      # layer-0 gather supertile (dst tiles)
STE = 2                      # layer-1 gather supertile
SD = 8                       # phase-D node tiles per supertile
AGCH = [12, 12, 12, 13]      # AllGather chunk sizes in dst tiles
AGB = np.cumsum([0] + AGCH)  # [0,12,24,36,49]
CHB = [0, 1536, 3072, 4608, 6250]   # chunk col boundaries

_cache = {}


# --------------------------------------------------------------------------
# host-side preparation
# --------------------------------------------------------------------------

def _wrap_idx(idx):
    """[n] int -> [128, n//16] int16 wrapped gather-index layout."""
    n = idx.shape[0]
    assert n % 16 == 0
    w = idx.reshape(n // 16, 16).T.astype(np.int16)
    return np.tile(w, (8, 1))


def _prep_edges(src, dst):
    cores = []
    for c in range(NCORES):
        m = (dst >= c * NLOC) & (dst < (c + 1) * NLOC)
        s = src[m].astype(np.int64)
        d = dst[m].astype(np.int64) - c * NLOC
        order = np.argsort(d, kind="stable")
        s, d = s[order], d[order]
        s_rot = (s - c * NLOC) % N
        tiles = []
        for t in range(LT):
            sel = (d >= t * 128) & (d < (t + 1) * 128)
            st, dt = s_rot[sel], d[sel] - t * 128
            lo = st < SPLIT
            tiles.append((st[lo], dt[lo], st[~lo] - SPLIT, dt[~lo]))
        cores.append(tiles)
    nl = max(len(t[0]) for tl in cores for t in tl)
    nh = max(len(t[2]) for tl in cores for t in tl)
    NL = max(1, (nl + 127) // 128)
    NH = max(1, (nh + 127) // 128)
    CH = NL + NH

    out = []
    for c in range(NCORES):
        ilb = np.zeros((LT, 128, NL * 8), np.int16)
        ihb = np.zeros((LT, 128, NH * 8), np.int16)
        aib = np.zeros((LT, 128, CH * 8), np.int16)
        drb = np.zeros((LT, 128, CH), np.int16)
        for t in range(LT):
            sl, dl, sh, dh = cores[c][t]
            il = np.zeros(NL * 128, np.int64)
            il[: len(sl)] = sl
            ih = np.zeros(NH * 128, np.int64)
            ih[: len(sh)] = sh
            ai = np.zeros(CH * 128, np.int64)
            ai[: len(dl)] = t * 128 + dl
            ai[NL * 128: NL * 128 + len(dh)] = t * 128 + dh
            ilb[t] = _wrap_idx(il)
            ihb[t] = _wrap_idx(ih)
            aib[t] = _wrap_idx(ai)
            rl = np.full(NL * 128, SENT)
            rl[: len(dl)] = dl
            rh = np.full(NH * 128, SENT)
            rh[: len(dh)] = dh
            r = np.concatenate([rl, rh]).reshape(CH, 128).T
            drb[t] = r.astype(np.float16).view(np.int16)
        epack = np.concatenate(
            [ilb.transpose(1, 0, 2).reshape(128, -1),
             ihb.transpose(1, 0, 2).reshape(128, -1),
             aib.transpose(1, 0, 2).reshape(128, -1),
             drb.transpose(1, 0, 2).reshape(128, -1)], axis=1)
        out.append(dict(epack=np.ascontiguousarray(epack)))
    return NL, NH, out


def _prep_inputs(x, edge_index, W0, a_src0, a_dst0, b0, W1, a_src1, a_dst1,
                 b1):
    src = np.asarray(edge_index[0]).astype(np.int64)
    dst = np.asarray(edge_index[1]).astype(np.int64)
    NL, NH, edata = _prep_edges(src, dst)

    def bd(a):  # [H, D] -> blockdiag [H*D, H]
        a = np.asarray(a, np.float32)
        H, D = a.shape
        m = np.zeros((H * D, H), np.float32)
        for h in range(H):
            m[h * D:(h + 1) * D, h] = a[h]
        return m

    W0 = np.asarray(W0, np.float32)
    W1 = np.asarray(W1, np.float32)
    W0a = np.concatenate([W0 @ bd(a_src0), W0 @ bd(a_dst0)], 1)  # [256, 16]
    # head-innermost feature interleave: new col d*8+h <- old col h*D+d
    perm0 = np.array([(f % 8) * 16 + f // 8 for f in range(128)])
    perm1 = np.array([(f % 8) * 64 + f // 8 for f in range(512)])
    W0cat = np.concatenate([W0[:, perm0], W0a], 1)               # [256, 144]
    W1a = np.concatenate([W1 @ bd(a_src1), W1 @ bd(a_dst1)], 1)  # [128, 16]

    x = np.asarray(x, np.float32)
    ident = np.eye(128, dtype=np.float16)
    colio = np.tile(np.arange(128, dtype=np.float16)[None, :], (128, 1))
    b0b = np.tile(np.asarray(b0, np.float32)[None, :], (128, 1))
    b1b = np.tile(np.asarray(b1, np.float32)[None, :], (128, 1))

    in_maps = []
    for c in range(NCORES):
        rot = np.roll(np.arange(N), -c * NLOC)
        xr = np.zeros((GROWS, NFEAT), np.float16)
        xr[:N] = x[rot].astype(np.float16)
        xtt = xr.reshape(GROWS // 128, 128, 2, 128).transpose(0, 3, 2, 1)
        m = dict(
            xT=np.ascontiguousarray(xtt),
            W0=np.ascontiguousarray(
                W0cat.astype(np.float16).reshape(2, 128, NHID + 16)),
            W1=np.ascontiguousarray(W1[perm0][:, perm1].astype(np.float16)),
            W1a=np.ascontiguousarray(W1a[perm0].astype(np.float16)),
            b0b=np.ascontiguousarray(b0b[:, perm0]), b1b=b1b,
            ident=ident, colio=colio,
            **edata[c],
        )
        in_maps.append(m)
    return NL, NH, in_maps


# --------------------------------------------------------------------------
# device program
# --------------------------------------------------------------------------

def build(NL, NH):
    CH = NL + NH
    HID16 = NHID + 16
    ILB = 0                       # epack col offsets (int16 cols)
    IHB = ILB + LT * NL * 8
    AIB = IHB + LT * NH * 8
    DRB = AIB + LT * CH * 8
    TOT = DRB + LT * CH

    nc = bacc.Bacc("TRN2")
    xT = nc.dram_tensor("xT", [GT, 128, 2, 128], F16, kind="ExternalInput")
    W0i = nc.dram_tensor("W0", [2, 128, HID16], F16, kind="ExternalInput")
    W1i = nc.dram_tensor("W1", [NHID, 512], F16, kind="ExternalInput")
    W1ai = nc.dram_tensor("W1a", [NHID, 16], F16, kind="ExternalInput")
    b0bi = nc.dram_tensor("b0b", [128, NHID], F32, kind="ExternalInput")
    b1bi = nc.dram_tensor("b1b", [128, NCLASS], F32, kind="ExternalInput")
    identi = nc.dram_tensor("ident", [128, 128], F16, kind="ExternalInput")
    colioi = nc.dram_tensor("colio", [128, 128], F16, kind="ExternalInput")
    epacki = nc.dram_tensor("epack", [128, TOT], I16, kind="ExternalInput")
    out = nc.dram_tensor("out", [NLOC, NCLASS], F32, kind="ExternalOutput")

    with TileContext(nc) as tc, ExitStack() as stk:
        dpool = stk.enter_context(
            tc.tile_pool(name="dram", bufs=1, space="DRAM"))
        t0lo = dpool.tile([SPLIT, T0W], F16, tag="t0lo")
        t0hi = dpool.tile([GROWS - SPLIT, T0W], F16, tag="t0hi")
        t0ad = dpool.tile([LT * 128, 128], F16, tag="t0ad")
        t1lo = dpool.tile([SPLIT, T1W], F16, tag="t1lo")
        t1hi = dpool.tile([GROWS - SPLIT, T1W], F16, tag="t1hi")
        t1ad = dpool.tile([LT * 128, 128], F16, tag="t1ad")
        agin = []
        agout = []
        for k in range(4):
            agin_k = dpool.tile([128, AGCH[k] * 128], I8, tag=f"agin{k}",
                                name=f"agin{k}")
            agout_k = dpool.tile([NCORES * 128, AGCH[k] * 128], I8,
                                 tag=f"agout{k}", addr_space="Shared",
                                 name=f"agout{k}")
            agin.append(agin_k)
            agout.append(agout_k)

        cpool = stk.enter_context(tc.tile_pool(name="const", bufs=1))
        W0s = cpool.tile([128, 2, HID16], F16)
        nc.sync.dma_start(out=W0s[:], in_=W0i.rearrange("k p n -> p k n"))
        W1s = cpool.tile([128, 512], F16)
        nc.sync.dma_start(out=W1s[:], in_=W1i[:])
        W1as = cpool.tile([128, 16], F16)
        nc.sync.dma_start(out=W1as[:], in_=W1ai[:])
        b0s = cpool.tile([128, NHID], F32)
        nc.sync.dma_start(out=b0s[:], in_=b0bi[:])
        b1s = cpool.tile([128, NCLASS], F32)
        nc.sync.dma_start(out=b1s[:], in_=b1bi[:])
        idents = cpool.tile([128, 128], F16)
        nc.sync.dma_start(out=idents[:], in_=identi[:])
        colios = cpool.tile([128, 128], F16)
        nc.sync.dma_start(out=colios[:], in_=colioi[:])
        adball = cpool.tile([128, LT * 8], F16)
        adbal2 = cpool.tile([128, LT * 8], F16)
        epS = cpool.tile([128, TOT], I16)
        nc.scalar.dma_start(out=epS[:], in_=epacki[:])

        regs = {}

        def reg(n):
            if n not in regs:
                regs[n] = nc.gpsimd.to_reg(n)
            return regs[n]

        # ---------------- phase A: layer-0 tables (replicated) ------------
        with ExitStack() as pa:
            xp = pa.enter_context(tc.tile_pool(name="pa_x", bufs=3))
            pp = pa.enter_context(
                tc.tile_pool(name="pa_ps", bufs=1, space="PSUM"))
            rp = pa.enter_context(tc.tile_pool(name="pa_row", bufs=3))
            for gg in range(GT // 8):
                xa = xp.tile([128, 8, 2, 128], F16, tag="xa")
                eng = nc.sync if gg % 2 else nc.scalar
                eng.dma_start(
                    out=xa[:],
                    in_=xT[8 * gg:8 * gg + 8].rearrange(
                        "g p k j -> p g k j"))
                row = rp.tile([128, 8, 136], F16, tag="row")
                for g2 in range(8):
                    ps = pp.tile([128, HID16], F32, tag=f"ps{g2 % 4}")
                    for k in range(2):
                        nc.tensor.matmul(ps[:], xa[:, g2, k, :],
                                         W0s[:, k, :],
                                         start=(k == 0), stop=(k == 1))
                    nc.vector.tensor_copy(row[:, g2, :], ps[:, 0:136])
                    g = 8 * gg + g2
                    if g < LT:
                        nc.vector.tensor_copy(
                            adball[:, g * 8:(g + 1) * 8], ps[:, 136:144])
                g0 = gg * 1024
                weng = nc.scalar if gg % 2 else nc.sync
                if g0 + 1024 <= SPLIT:
                    weng.dma_start(
                        out=t0lo[g0:g0 + 1024, 0:136]
                        .rearrange("(g p) w -> p g w", p=128),
                        in_=row[:])
                elif g0 >= SPLIT:
                    o = g0 - SPLIT
                    weng.dma_start(
                        out=t0hi[o:o + 1024, 0:136]
                        .rearrange("(g p) w -> p g w", p=128),
                        in_=row[:])
                else:  # straddles the split (gg == 24)
                    nlo = (SPLIT - g0) // 128
                    weng.dma_start(
                        out=t0lo[g0:SPLIT, 0:136]
                        .rearrange("(g p) w -> p g w", p=128),
                        in_=row[:, 0:nlo])
                    weng.dma_start(
                        out=t0hi[0:1024 - (SPLIT - g0), 0:136]
                        .rearrange("(g p) w -> p g w", p=128),
                        in_=row[:, nlo:8])
            nc.sync.dma_start(
                out=t0ad[:, 0:8].rearrange("(t p) w -> p t w", p=128),
                in_=adball[:].rearrange("p (t w) -> p t w", w=8))

        # ---------------- shared edge phase -------------------------------
        def edge_phase(layer, ST, tbl_lo, tbl_hi, tblad, fdim, post_fn,
                       agin_hook):
            D = fdim // HEADS
            trow = T1W if layer else T0W
            nst = (LT + ST - 1) // ST
            with ExitStack() as pb:
                gp = pb.enter_context(
                    tc.tile_pool(name=f"gg{layer}", bufs=2))
                apl = pb.enter_context(
                    tc.tile_pool(name=f"ga{layer}", bufs=2))
                incp = pb.enter_context(
                    tc.tile_pool(name=f"ic{layer}", bufs=3))
                rp2 = pb.enter_context(
                    tc.tile_pool(name=f"rh{layer}", bufs=3))
                exq = pb.enter_context(
                    tc.tile_pool(name=f"ex{layer}", bufs=3))
                pp2 = pb.enter_context(
                    tc.tile_pool(name=f"ps{layer}", bufs=3, space="PSUM"))
                op = pb.enter_context(
                    tc.tile_pool(name=f"po{layer}", bufs=3))
                for st in range(nst):
                    a, b = ST * st, min(ST * st + ST, LT)
                    nt = b - a
                    glo = gp.tile([128, ST * NL, trow], F16, tag="glo")
                    nc.gpsimd.dma_gather(
                        glo[:, 0:nt * NL, :], tbl_lo[:],
                        epS[:, ILB + a * NL * 8:ILB + b * NL * 8],
                        nt * NL * 128, reg(nt * NL * 128), trow)
                    ghi = gp.tile([128, ST * NH, trow], F16, tag="ghi")
                    nc.gpsimd.dma_gather(
                        ghi[:, 0:nt * NH, :], tbl_hi[:],
                        epS[:, IHB + a * NH * 8:IHB + b * NH * 8],
                        nt * NH * 128, reg(nt * NH * 128), trow)
                    ga = apl.tile([128, ST * CH, 128], F16, tag="ga")
                    nc.gpsimd.dma_gather(
                        ga[:, 0:nt * CH, :], tblad[:],
                        epS[:, AIB + a * CH * 8:AIB + b * CH * 8],
                        nt * CH * 128, reg(nt * CH * 128), 128)
                    for t in range(a, b):
                        i = t - a
                        dr = epS[:, DRB + t * CH:DRB + (t + 1) * CH]\
                            .bitcast(F16)
                        inc = incp.tile([128, CH, 128], F16, tag="inc")
                        ieng = nc.vector if t % 2 else nc.gpsimd
                        ieng.tensor_tensor(
                            out=inc[:],
                            in0=dr.unsqueeze(-1)
                            .broadcast_to([128, CH, 128]),
                            in1=colios[:].unsqueeze(1)
                            .broadcast_to([128, CH, 128]),
                            op=mybir.AluOpType.is_equal)
                        EX = exq.tile([128, CH, 8], F16, tag="EX")
                        nc.vector.tensor_tensor(
                            out=EX[:, 0:NL, :],
                            in0=glo[:, i * NL:(i + 1) * NL,
                                    fdim:fdim + 8],
                            in1=ga[:, i * CH:i * CH + NL, 0:8],
                            op=mybir.AluOpType.add)
                        nc.vector.tensor_tensor(
                            out=EX[:, NL:CH, :],
                            in0=ghi[:, i * NH:(i + 1) * NH,
                                    fdim:fdim + 8],
                            in1=ga[:, i * CH + NL:(i + 1) * CH, 0:8],
                            op=mybir.AluOpType.add)
                        nc.scalar.activation(
                            EX[:], EX[:],
                            mybir.ActivationFunctionType.Prelu,
                            alpha=SLOPE)
                        nc.scalar.activation(
                            EX[:], EX[:],
                            mybir.ActivationFunctionType.Exp)
                        rw = fdim + 8 if layer == 0 else fdim
                        R = rp2.tile([128, CH, rw], F16, tag="R")
                        nc.vector.tensor_tensor(
                            out=R[:, 0:NL, 0:fdim]
                            .rearrange("p c (d h) -> p c d h", h=HEADS),
                            in0=glo[:, i * NL:(i + 1) * NL, 0:fdim]
                            .rearrange("p c (d h) -> p c d h", h=HEADS),
                            in1=EX[:, 0:NL].unsqueeze(2)
                            .broadcast_to([128, NL, D, HEADS]),
                            op=mybir.AluOpType.mult)
                        nc.vector.tensor_tensor(
                            out=R[:, NL:CH, 0:fdim]
                            .rearrange("p c (d h) -> p c d h", h=HEADS),
                            in0=ghi[:, i * NH:(i + 1) * NH, 0:fdim]
                            .rearrange("p c (d h) -> p c d h", h=HEADS),
                            in1=EX[:, NL:CH].unsqueeze(2)
                            .broadcast_to([128, NH, D, HEADS]),
                            op=mybir.AluOpType.mult)
                        if layer == 0:
                            # fused denominator columns
                            nc.vector.tensor_copy(
                                R[:, :, fdim:fdim + 8], EX[:])
                            P1 = pp2.tile([128, 136], F32, tag="P1")
                            for ch in range(CH):
                                nc.tensor.matmul(
                                    P1[:], inc[:, ch, :], R[:, ch, :],
                                    start=(ch == 0), stop=(ch == CH - 1))
                            post_fn(t, P1, None, op, pp2)
                        else:
                            P1 = pp2.tile([128, 512], F32, tag="P1")
                            for ch in range(CH):
                                nc.tensor.matmul(
                                    P1[:], inc[:, ch, :], R[:, ch, :],
                                    start=(ch == 0), stop=(ch == CH - 1))
                            P2 = pp2.tile([128, 8], F32, tag="P2")
                            for ch in range(CH):
                                nc.tensor.matmul(
                                    P2[:], inc[:, ch, :], EX[:, ch, :],
                                    start=(ch == 0), stop=(ch == CH - 1))
                            post_fn(t, P1, P2, op, pp2)
                        if agin_hook is not None:
                            agin_hook(t)

        # ---- L0 post: softmax-div, +b0, ELU, transpose, f8, store --------
        def post0(t, P1, _, op, pp2):
            rows = 128 if t < LT - 1 else LAST_ROWS
            k = min(int(t) // 12, 3)
            col0 = (t - AGB[k]) * 128
            r8 = op.tile([128, 8], F32, tag="r8")
            nc.vector.tensor_scalar_add(r8[:], P1[:, 128:136], 1e-16)
            nc.vector.reciprocal(r8[:], r8[:])
            z = op.tile([128, NHID], F32, tag="z")
            nc.vector.tensor_tensor(
                out=z[:].rearrange("p (d h) -> p d h", h=HEADS),
                in0=P1[:, 0:128].rearrange("p (d h) -> p d h", h=HEADS),
                in1=r8[:].unsqueeze(1).broadcast_to([128, 16, HEADS]),
                op=mybir.AluOpType.mult)
            nc.vector.tensor_tensor(out=z[:], in0=z[:], in1=b0s[:],
                                    op=mybir.AluOpType.add)
            zm = op.tile([128, NHID], F32, tag="zm")
            nc.vector.tensor_scalar_min(zm[:], z[:], 0.0)
            nc.scalar.activation(zm[:], zm[:],
                                 mybir.ActivationFunctionType.Exp)
            zp = op.tile([128, NHID], F32, tag="zp")
            nc.vector.tensor_scalar_max(zp[:], z[:], 0.0)
            nc.vector.tensor_tensor(out=zp[:], in0=zp[:], in1=zm[:],
                                    op=mybir.AluOpType.add)
            h1 = op.tile([128, NHID], F16, tag="h1")
            nc.vector.tensor_scalar_add(h1[:], zp[:], -1.0)
            pst = pp2.tile([128, 128], F16, tag="pst")
            nc.tensor.transpose(pst[:], h1[:], idents[:])
            h8 = op.tile([128, 128], I8, tag="h8")
            nc.vector.tensor_copy(h8[:].bitcast(F8), pst[:])
            nc.sync.dma_start(
                out=agin[k][:, col0:col0 + rows], in_=h8[:, 0:rows])

        # AllGather chunks fire as soon as their agin columns are complete
        def agin_hook(t):
            for k in range(4):
                if t == AGB[k + 1] - 1:
                    nc.gpsimd.collective_compute(
                        "AllGather", mybir.AluOpType.bypass,
                        replica_groups=[list(range(NCORES))],
                        ins=[agin[k][:]], outs=[agout[k][:]])

        edge_phase(0, STB, t0lo, t0hi, t0ad, NHID, post0, agin_hook)

        pid = nc.partition_id(engines=[mybir.EngineType.SP])
        sregs = [nc.sync.snap(((j + pid) % NCORES) * 128)
                 for j in range(NCORES)]

        # ---------------- phase D: layer-1 tables -------------------------
        ngt = (N + 127) // 128   # 391
        sts = []
        for si in range((ngt + SD - 1) // SD):
            r0 = si * SD * 128
            r1 = min(r0 + SD * 128, ngt * 128)
            r1c = min(r1, N)
            dep = -1
            r = r0
            while r < r1c:
                j = r // NLOC
                cl = r - j * NLOC
                k = next(kk for kk in range(4) if cl < CHB[kk + 1])
                end = min(r1c, j * NLOC + CHB[k + 1])
                if j > 0:
                    dep = max(dep, k)
                r = end
            sts.append((dep, si, r0, r1, r1c))
        sts.sort()

        with ExitStack() as pd:
            xp1 = pd.enter_context(tc.tile_pool(name="pd_x", bufs=3))
            pp1 = pd.enter_context(
                tc.tile_pool(name="pd_ps", bufs=2, space="PSUM"))
            rp1 = pd.enter_context(tc.tile_pool(name="pd_row", bufs=3))
            for _, si, r0, r1, r1c in sts:
                nt = (r1 - r0) // 128
                hx = xp1.tile([128, SD * 128], I8, tag="hx")
                r = r0
                while r < r1c:
                    j = r // NLOC
                    cl = r - j * NLOC
                    k = next(kk for kk in range(4) if cl < CHB[kk + 1])
                    end = min(r1c, j * NLOC + CHB[k + 1])
                    seg = end - r
                    if j == 0:
                        nc.sync.dma_start(
                            out=hx[:, r - r0:r - r0 + seg],
                            in_=agin[k][:, cl - CHB[k]:cl - CHB[k] + seg])
                    else:
                        nc.sync.dma_start(
                            out=hx[:, r - r0:r - r0 + seg],
                            in_=agout[k][bass.ds(sregs[j], 128),
                                         cl - CHB[k]:cl - CHB[k] + seg])
                    r = end
                if r1c < r1:
                    nc.vector.memset(hx[:, r1c - r0:r1 - r0], 0)
                row = rp1.tile([128, SD, 520], F16, tag="row")
                for g2 in range(nt):
                    ps = pp1.tile([128, 512], F32, tag=f"ps{g2 % 2}")
                    nc.tensor.matmul(ps[:],
                                     hx[:, g2 * 128:(g2 + 1) * 128]
                                     .bitcast(F8),
                                     W1s[:], start=True, stop=True)
                    psa = pp1.tile([128, 16], F32, tag=f"psa{g2 % 2}")
                    nc.tensor.matmul(psa[:],
                                     hx[:, g2 * 128:(g2 + 1) * 128]
                                     .bitcast(F8),
                                     W1as[:], start=True, stop=True)
                    if g2 % 2:
                        nc.scalar.copy(row[:, g2, 0:512], ps[:])
                    else:
                        nc.vector.tensor_copy(row[:, g2, 0:512], ps[:])
                    nc.vector.tensor_copy(row[:, g2, 512:520],
                                          psa[:, 0:8])
                    g = si * SD + g2
                    if g < LT:
                        nc.vector.tensor_copy(
                            adbal2[:, g * 8:(g + 1) * 8], psa[:, 8:16])
                weng = nc.scalar if si % 2 else nc.sync
                if r1 <= SPLIT:
                    weng.dma_start(
                        out=t1lo[r0:r1, 0:520]
                        .rearrange("(g p) w -> p g w", p=128),
                        in_=row[:, 0:nt, :])
                elif r0 >= SPLIT:
                    weng.dma_start(
                        out=t1hi[r0 - SPLIT:r1 - SPLIT, 0:520]
                        .rearrange("(g p) w -> p g w", p=128),
                        in_=row[:, 0:nt, :])
                else:
                    nlo = (SPLIT - r0) // 128
                    weng.dma_start(
                        out=t1lo[r0:SPLIT, 0:520]
                        .rearrange("(g p) w -> p g w", p=128),
                        in_=row[:, 0:nlo, :])
                    weng.dma_start(
                        out=t1hi[0:r1 - SPLIT, 0:520]
                        .rearrange("(g p) w -> p g w", p=128),
                        in_=row[:, nlo:nt, :])
            nc.sync.dma_start(
                out=t1ad[:, 0:8].rearrange("(t p) w -> p t w", p=128),
                in_=adbal2[:].rearrange("p (t w) -> p t w", w=8))

        # ---------------- phase E: layer-1 edges + epilogue ---------------
        fpool = stk.enter_context(tc.tile_pool(name="fin", bufs=1))
        zbig = fpool.tile([128, LT * NCLASS], F32)
        nmxb = fpool.tile([128, LT], F32)
        seb = fpool.tile([128, LT], F32)

        def post1(t, P1, P2, op, pp2):
            r8 = op.tile([128, 8], F32, tag="r8")
            nc.vector.tensor_scalar_add(r8[:], P2[:], 1e-16)
            nc.vector.reciprocal(r8[:], r8[:])
            nc.vector.tensor_scalar_mul(r8[:], r8[:], 1.0 / HEADS)
            zw = op.tile([128, 512], F32, tag="zw")
            nc.vector.tensor_tensor(
                out=zw[:].rearrange("p (d h) -> p d h", h=HEADS),
                in0=P1[:].rearrange("p (d h) -> p d h", h=HEADS),
                in1=r8[:].unsqueeze(1).broadcast_to([128, 64, HEADS]),
                op=mybir.AluOpType.mult)
            z = zbig[:, t * NCLASS:(t + 1) * NCLASS]
            nc.vector.reduce_sum(
                z, zw[:].rearrange("p (d h) -> p d h", h=HEADS),
                axis=mybir.AxisListType.X)
            nc.vector.tensor_tensor(out=z, in0=z, in1=b1s[:],
                                    op=mybir.AluOpType.add)
            nmx = nmxb[:, t:t + 1]
            nc.vector.reduce_max(nmx, z, axis=mybir.AxisListType.X,
                                 negate=True)
            ez = op.tile([128, NCLASS], F32, tag="ez")
            nc.scalar.activation(ez[:], z,
                                 mybir.ActivationFunctionType.Exp,
                                 bias=nmx, accum_out=seb[:, t:t + 1])

        edge_phase(1, STE, t1lo, t1hi, t1ad, 512, post1, None)

        # batched log-softmax tail
        nc.scalar.activation(seb[:], seb[:],
                             mybir.ActivationFunctionType.Ln)
        nc.vector.tensor_tensor(
            out=zbig[:].rearrange("p (t c) -> p t c", c=NCLASS),
            in0=zbig[:].rearrange("p (t c) -> p t c", c=NCLASS),
            in1=nmxb[:].unsqueeze(-1).broadcast_to([128, LT, NCLASS]),
            op=mybir.AluOpType.add)
        nc.vector.tensor_tensor(
            out=zbig[:].rearrange("p (t c) -> p t c", c=NCLASS),
            in0=zbig[:].rearrange("p (t c) -> p t c", c=NCLASS),
            in1=seb[:].unsqueeze(-1).broadcast_to([128, LT, NCLASS]),
            op=mybir.AluOpType.subtract)
        nfull = (LT - 1) * 128
        nc.sync.dma_start(
            out=out[0:nfull, :].rearrange("(t p) c -> p t c", p=128),
            in_=zbig[:].rearrange("p (t c) -> p t c", c=NCLASS)
            [:, 0:LT - 1, :])
        nc.sync.dma_start(
            out=out[nfull:nfull + LAST_ROWS, :],
            in_=zbig[0:LAST_ROWS, (LT - 1) * NCLASS:LT * NCLASS])

    nc.compile()
    return nc


# --------------------------------------------------------------------------
# entry point
# --------------------------------------------------------------------------

def kernel(**inputs) -> np.ndarray:
    NLk, NHk, in_maps = _prep_inputs(**inputs)
    key = (NLk, NHk)
    if key not in _cache:
        _cache[key] = build(NLk, NHk)
    nc = _cache[key]
    res = run_bass_kernel_spmd(nc, in_maps, list(range(NCORES)))
    return np.concatenate([res.results[c]["out"] for c in range(NCORES)], 0)


# revision 8
# speedup vs baseline: 1.1721x; 1.0503x over previous
"""2-layer GAT (nn_GAT_31490700214331) on 8 Trainium2 NeuronCores.

Strategy (dst-sharded, SPMD, per-core-rotated node layout) — v2:
  - Nodes block-partitioned: core c owns nodes [c*6250, (c+1)*6250); every
    table on core c uses a ROTATED row order (node n at row (n - c*6250)
    mod 50000) so one SPMD program serves all cores.
  - Phase A (replicated): h0 = x @ [W0 | W0·a_src | W0·a_dst] for all
    nodes; rows [h0|as0] land in the gather tables t0lo/t0hi (512 B rows),
    dst-alphas accumulate in SBUF and are written to t0ad in one DMA.
  - Phase B: per group of 4 dst tiles, three batched dma_gathers (src rows
    lo/hi + per-edge dst-alpha); edge softmax (safe without segment-max)
    and aggregation run as 128x128 incidence matmuls; denominators ride
    as 8 fused psum columns.  Incidence builds alternate DVE/GpSimd.
  - The hidden state is ELU'd, transposed, quantized to f8e4 and
    AllGather'd in 4 column chunks that overlap phase B's tail and
    phase D's head (COLLECTIVE_CORES runs concurrently with compute).
  - Phase D: supertiles of 8 node tiles, ordered by which AllGather chunk
    they need (own-core rows first, straight from local agin); the f8
    hidden state feeds mixed-precision matmuls with W1/W1a; rows
    [h1|as1] go to t1lo/t1hi (1280 B rows), dst-alphas to t1ad.
  - Phase E: like B with 640-col rows, separate denominator chain, and a
    head-mean + batched log_softmax epilogue.
  - alpha projections fold into the weight matmuls on the host:
    h @ blockdiag(a) == x @ (W @ blockdiag(a)).

Self-contained: call kernel(**inputs) with the full-problem arrays.
"""
import numpy as np
from contextlib import ExitStack

import concourse.bacc as bacc
import concourse.bass as bass
import concourse.mybir as mybir
from concourse.tile import TileContext
from concourse.bass_utils import run_bass_kernel_spmd

F16 = mybir.dt.float16
F32 = mybir.dt.float32
F8 = mybir.dt.float8e4
I16 = mybir.dt.int16
I8 = mybir.dt.int8

N = 50000
NFEAT = 256
NHID = 128
NCLASS = 64
HEADS = 8
SLOPE = 0.2
NCORES = 8
NLOC = N // NCORES           # 6250
LT = (NLOC + 127) // 128     # 49 local dst tiles
LAST_ROWS = NLOC - (LT - 1) * 128   # 106 rows in the last tile
GT = 392                     # global node tiles (392*128 = 50176)
GROWS = GT * 128
SPLIT = 25088                # low/high gather-table split (196 tiles)
SENT = 300.0                 # dst_rel sentinel for padding slots
T0W = 256                    # t0 row: [h0(128)|as0(8)|junk] f16
T1W = 640                    # t1 row: [h1(512)|as1(8)|junk] f16
STB = 4                      # layer-0 gather supertile (dst tiles)
STE = 2                      # layer-1 gather supertile
SD = 8                       # phase-D node tiles per supertile
AGCH = [12, 12, 12, 13]      # AllGather chunk sizes in dst tiles
AGB = np.cumsum([0] + AGCH)  # [0,12,24,36,49]
CHB = [0, 1536, 3072, 4608, 6250]   # chunk col boundaries

_cache = {}


# --------------------------------------------------------------------------
# host-side preparation
# --------------------------------------------------------------------------

def _wrap_idx(idx):
    """[n] int -> [128, n//16] int16 wrapped gather-index layout."""
    n = idx.shape[0]
    assert n % 16 == 0
    w = idx.reshape(n // 16, 16).T.astype(np.int16)
    return np.tile(w, (8, 1))


def _prep_edges(src, dst):
    cores = []
    for c in range(NCORES):
        m = (dst >= c * NLOC) & (dst < (c + 1) * NLOC)
        s = src[m].astype(np.int64)
        d = dst[m].astype(np.int64) - c * NLOC
        order = np.argsort(d, kind="stable")
        s, d = s[order], d[order]
        s_rot = (s - c * NLOC) % N
        tiles = []
        for t in range(LT):
            sel = (d >= t * 128) & (d < (t + 1) * 128)
            st, dt = s_rot[sel], d[sel] - t * 128
            lo = st < SPLIT
            tiles.append((st[lo], dt[lo], st[~lo] - SPLIT, dt[~lo]))
        cores.append(tiles)
    nl = max(len(t[0]) for tl in cores for t in tl)
    nh = max(len(t[2]) for tl in cores for t in tl)
    NL = max(1, (nl + 127) // 128)
    NH = max(1, (nh + 127) // 128)
    CH = NL + NH

    out = []
    for c in range(NCORES):
        ilb = np.zeros((LT, 128, NL * 8), np.int16)
        ihb = np.zeros((LT, 128, NH * 8), np.int16)
        aib = np.zeros((LT, 128, CH * 8), np.int16)
        drb = np.zeros((LT, 128, CH), np.int16)
        for t in range(LT):
            sl, dl, sh, dh = cores[c][t]
            il = np.zeros(NL * 128, np.int64)
            il[: len(sl)] = sl
            ih = np.zeros(NH * 128, np.int64)
            ih[: len(sh)] = sh
            ai = np.zeros(CH * 128, np.int64)
            ai[: len(dl)] = t * 128 + dl
            ai[NL * 128: NL * 128 + len(dh)] = t * 128 + dh
            ilb[t] = _wrap_idx(il)
            ihb[t] = _wrap_idx(ih)
            aib[t] = _wrap_idx(ai)
            rl = np.full(NL * 128, SENT)
            rl[: len(dl)] = dl
            rh = np.full(NH * 128, SENT)
            rh[: len(dh)] = dh
            r = np.concatenate([rl, rh]).reshape(CH, 128).T
            drb[t] = r.astype(np.float16).view(np.int16)
        epack = np.concatenate(
            [ilb.transpose(1, 0, 2).reshape(128, -1),
             ihb.transpose(1, 0, 2).reshape(128, -1),
             aib.transpose(1, 0, 2).reshape(128, -1),
             drb.transpose(1, 0, 2).reshape(128, -1)], axis=1)
        out.append(dict(epack=np.ascontiguousarray(epack)))
    return NL, NH, out


def _prep_inputs(x, edge_index, W0, a_src0, a_dst0, b0, W1, a_src1, a_dst1,
                 b1):
    src = np.asarray(edge_index[0]).astype(np.int64)
    dst = np.asarray(edge_index[1]).astype(np.int64)
    NL, NH, edata = _prep_edges(src, dst)

    def bd(a):  # [H, D] -> blockdiag [H*D, H]
        a = np.asarray(a, np.float32)
        H, D = a.shape
        m = np.zeros((H * D, H), np.float32)
        for h in range(H):
            m[h * D:(h + 1) * D, h] = a[h]
        return m

    W0 = np.asarray(W0, np.float32)
    W1 = np.asarray(W1, np.float32)
    W0a = np.concatenate([W0 @ bd(a_src0), W0 @ bd(a_dst0)], 1)  # [256, 16]
    # head-innermost feature interleave: new col d*8+h <- old col h*D+d
    perm0 = np.array([(f % 8) * 16 + f // 8 for f in range(128)])
    perm1 = np.array([(f % 8) * 64 + f // 8 for f in range(512)])
    W0cat = np.concatenate([W0[:, perm0], W0a], 1)               # [256, 144]
    W1a = np.concatenate([W1 @ bd(a_src1), W1 @ bd(a_dst1)], 1)  # [128, 16]

    x = np.asarray(x, np.float32)
    ident = np.eye(128, dtype=np.float16)
    CH = NL + NH
    colio = np.tile(np.repeat(np.arange(128, dtype=np.float16), CH)[None, :],
                    (128, 1))
    b0b = np.tile(np.asarray(b0, np.float32)[None, :], (128, 1))
    b1b = np.tile(np.asarray(b1, np.float32)[None, :], (128, 1))

    in_maps = []
    for c in range(NCORES):
        rot = np.roll(np.arange(N), -c * NLOC)
        xr = np.zeros((GROWS, NFEAT), np.float16)
        xr[:N] = x[rot].astype(np.float16)
        xtt = xr.reshape(GROWS // 128, 128, 2, 128).transpose(0, 3, 2, 1)
        m = dict(
            xT=np.ascontiguousarray(xtt),
            W0=np.ascontiguousarray(
                W0cat.astype(np.float16).reshape(2, 128, NHID + 16)),
            W1=np.ascontiguousarray(W1[perm0][:, perm1].astype(np.float16)),
            W1a=np.ascontiguousarray(W1a[perm0].astype(np.float16)),
            b0b=np.ascontiguousarray(b0b[:, perm0]), b1b=b1b,
            ident=ident, colio=colio,
            **edata[c],
        )
        in_maps.append(m)
    return NL, NH, in_maps


# --------------------------------------------------------------------------
# device program
# --------------------------------------------------------------------------

def build(NL, NH):
    CH = NL + NH
    HID16 = NHID + 16
    ILB = 0                       # epack col offsets (int16 cols)
    IHB = ILB + LT * NL * 8
    AIB = IHB + LT * NH * 8
    DRB = AIB + LT * CH * 8
    TOT = DRB + LT * CH

    nc = bacc.Bacc("TRN2")
    xT = nc.dram_tensor("xT", [GT, 128, 2, 128], F16, kind="ExternalInput")
    W0i = nc.dram_tensor("W0", [2, 128, HID16], F16, kind="ExternalInput")
    W1i = nc.dram_tensor("W1", [NHID, 512], F16, kind="ExternalInput")
    W1ai = nc.dram_tensor("W1a", [NHID, 16], F16, kind="ExternalInput")
    b0bi = nc.dram_tensor("b0b", [128, NHID], F32, kind="ExternalInput")
    b1bi = nc.dram_tensor("b1b", [128, NCLASS], F32, kind="ExternalInput")
    identi = nc.dram_tensor("ident", [128, 128], F16, kind="ExternalInput")
    colioi = nc.dram_tensor("colio", [128, 128 * CH], F16,
                            kind="ExternalInput")
    epacki = nc.dram_tensor("epack", [128, TOT], I16, kind="ExternalInput")
    out = nc.dram_tensor("out", [NLOC, NCLASS], F32, kind="ExternalOutput")

    with TileContext(nc) as tc, ExitStack() as stk:
        dpool = stk.enter_context(
            tc.tile_pool(name="dram", bufs=1, space="DRAM"))
        t0lo = dpool.tile([SPLIT, T0W], F16, tag="t0lo")
        t0hi = dpool.tile([GROWS - SPLIT, T0W], F16, tag="t0hi")
        t0ad = dpool.tile([LT * 128, 128], F16, tag="t0ad")
        t1lo = dpool.tile([SPLIT, T1W], F16, tag="t1lo")
        t1hi = dpool.tile([GROWS - SPLIT, T1W], F16, tag="t1hi")
        t1ad = dpool.tile([LT * 128, 128], F16, tag="t1ad")
        agin = []
        agout = []
        for k in range(4):
            agin_k = dpool.tile([128, AGCH[k] * 128], I8, tag=f"agin{k}",
                                name=f"agin{k}")
            agout_k = dpool.tile([NCORES * 128, AGCH[k] * 128], I8,
                                 tag=f"agout{k}", addr_space="Shared",
                                 name=f"agout{k}")
            agin.append(agin_k)
            agout.append(agout_k)

        cpool = stk.enter_context(tc.tile_pool(name="const", bufs=1))
        W0s = cpool.tile([128, 2, HID16], F16)
        nc.sync.dma_start(out=W0s[:], in_=W0i.rearrange("k p n -> p k n"))
        W1s = cpool.tile([128, 512], F16)
        nc.sync.dma_start(out=W1s[:], in_=W1i[:])
        W1as = cpool.tile([128, 16], F16)
        nc.sync.dma_start(out=W1as[:], in_=W1ai[:])
        b0s = cpool.tile([128, NHID], F32)
        nc.sync.dma_start(out=b0s[:], in_=b0bi[:])
        b1s = cpool.tile([128, NCLASS], F32)
        nc.sync.dma_start(out=b1s[:], in_=b1bi[:])
        idents = cpool.tile([128, 128], F16)
        nc.sync.dma_start(out=idents[:], in_=identi[:])
        colios = cpool.tile([128, 128 * CH], F16)
        nc.sync.dma_start(out=colios[:], in_=colioi[:])
        adball = cpool.tile([128, LT * 8], F16)
        adbal2 = cpool.tile([128, LT * 8], F16)
        epS = cpool.tile([128, TOT], I16)
        nc.scalar.dma_start(out=epS[:], in_=epacki[:])

        regs = {}

        def reg(n):
            if n not in regs:
                regs[n] = nc.gpsimd.to_reg(n)
            return regs[n]

        # ---------------- phase A: layer-0 tables (replicated) ------------
        with ExitStack() as pa:
            xp = pa.enter_context(tc.tile_pool(name="pa_x", bufs=3))
            pp = pa.enter_context(
                tc.tile_pool(name="pa_ps", bufs=1, space="PSUM"))
            rp = pa.enter_context(tc.tile_pool(name="pa_row", bufs=3))
            for gg in range(GT // 8):
                xa = xp.tile([128, 8, 2, 128], F16, tag="xa")
                eng = nc.sync if gg % 2 else nc.scalar
                eng.dma_start(
                    out=xa[:],
                    in_=xT[8 * gg:8 * gg + 8].rearrange(
                        "g p k j -> p g k j"))
                row = rp.tile([128, 8, 136], F16, tag="row")
                for g2 in range(8):
                    ps = pp.tile([128, HID16], F32, tag=f"ps{g2 % 4}")
                    for k in range(2):
                        nc.tensor.matmul(ps[:], xa[:, g2, k, :],
                                         W0s[:, k, :],
                                         start=(k == 0), stop=(k == 1))
                    nc.vector.tensor_copy(row[:, g2, :], ps[:, 0:136])
                    g = 8 * gg + g2
                    if g < LT:
                        nc.vector.tensor_copy(
                            adball[:, g * 8:(g + 1) * 8], ps[:, 136:144])
                g0 = gg * 1024
                weng = nc.scalar if gg % 2 else nc.sync
                if g0 + 1024 <= SPLIT:
                    weng.dma_start(
                        out=t0lo[g0:g0 + 1024, 0:136]
                        .rearrange("(g p) w -> p g w", p=128),
                        in_=row[:])
                elif g0 >= SPLIT:
                    o = g0 - SPLIT
                    weng.dma_start(
                        out=t0hi[o:o + 1024, 0:136]
                        .rearrange("(g p) w -> p g w", p=128),
                        in_=row[:])
                else:  # straddles the split (gg == 24)
                    nlo = (SPLIT - g0) // 128
                    weng.dma_start(
                        out=t0lo[g0:SPLIT, 0:136]
                        .rearrange("(g p) w -> p g w", p=128),
                        in_=row[:, 0:nlo])
                    weng.dma_start(
                        out=t0hi[0:1024 - (SPLIT - g0), 0:136]
                        .rearrange("(g p) w -> p g w", p=128),
                        in_=row[:, nlo:8])
            nc.sync.dma_start(
                out=t0ad[:, 0:8].rearrange("(t p) w -> p t w", p=128),
                in_=adball[:].rearrange("p (t w) -> p t w", w=8))

        # ---------------- shared edge phase -------------------------------
        def edge_phase(layer, ST, tbl_lo, tbl_hi, tblad, fdim, post_fn,
                       agin_hook):
            D = fdim // HEADS
            trow = T1W if layer else T0W
            nst = (LT + ST - 1) // ST
            with ExitStack() as pb:
                gp = pb.enter_context(
                    tc.tile_pool(name=f"gg{layer}", bufs=2))
                apl = pb.enter_context(
                    tc.tile_pool(name=f"ga{layer}", bufs=2))
                incp = pb.enter_context(
                    tc.tile_pool(name=f"ic{layer}", bufs=3))
                rp2 = pb.enter_context(
                    tc.tile_pool(name=f"rh{layer}", bufs=3))
                exq = pb.enter_context(
                    tc.tile_pool(name=f"ex{layer}", bufs=3))
                pp2 = pb.enter_context(
                    tc.tile_pool(name=f"ps{layer}", bufs=3, space="PSUM"))
                op = pb.enter_context(
                    tc.tile_pool(name=f"po{layer}", bufs=3))
                for st in range(nst):
                    a, b = ST * st, min(ST * st + ST, LT)
                    nt = b - a
                    glo = gp.tile([128, ST * NL, trow], F16, tag="glo")
                    nc.gpsimd.dma_gather(
                        glo[:, 0:nt * NL, :], tbl_lo[:],
                        epS[:, ILB + a * NL * 8:ILB + b * NL * 8],
                        nt * NL * 128, reg(nt * NL * 128), trow)
                    ghi = gp.tile([128, ST * NH, trow], F16, tag="ghi")
                    nc.gpsimd.dma_gather(
                        ghi[:, 0:nt * NH, :], tbl_hi[:],
                        epS[:, IHB + a * NH * 8:IHB + b * NH * 8],
                        nt * NH * 128, reg(nt * NH * 128), trow)
                    ga = apl.tile([128, ST * CH, 128], F16, tag="ga")
                    nc.gpsimd.dma_gather(
                        ga[:, 0:nt * CH, :], tblad[:],
                        epS[:, AIB + a * CH * 8:AIB + b * CH * 8],
                        nt * CH * 128, reg(nt * CH * 128), 128)
                    for t in range(a, b):
                        i = t - a
                        dr = epS[:, DRB + t * CH:DRB + (t + 1) * CH]\
                            .bitcast(F16)
                        inc = incp.tile([128, 128, CH], F16, tag="inc")
                        nc.vector.tensor_tensor(
                            out=inc[:],
                            in0=dr.unsqueeze(1)
                            .broadcast_to([128, 128, CH]),
                            in1=colios[:]
                            .rearrange("p (d c) -> p d c", c=CH),
                            op=mybir.AluOpType.is_equal)
                        EX = exq.tile([128, CH, 8], F16, tag="EX")
                        nc.vector.tensor_tensor(
                            out=EX[:, 0:NL, :],
                            in0=glo[:, i * NL:(i + 1) * NL,
                                    fdim:fdim + 8],
                            in1=ga[:, i * CH:i * CH + NL, 0:8],
                            op=mybir.AluOpType.add)
                        nc.vector.tensor_tensor(
                            out=EX[:, NL:CH, :],
                            in0=ghi[:, i * NH:(i + 1) * NH,
                                    fdim:fdim + 8],
                            in1=ga[:, i * CH + NL:(i + 1) * CH, 0:8],
                            op=mybir.AluOpType.add)
                        nc.scalar.activation(
                            EX[:], EX[:],
                            mybir.ActivationFunctionType.Prelu,
                            alpha=SLOPE)
                        nc.scalar.activation(
                            EX[:], EX[:],
                            mybir.ActivationFunctionType.Exp)
                        rw = fdim + 8 if layer == 0 else fdim
                        R = rp2.tile([128, CH, rw], F16, tag="R")
                        nc.vector.tensor_tensor(
                            out=R[:, 0:NL, 0:fdim]
                            .rearrange("p c (d h) -> p c d h", h=HEADS),
                            in0=glo[:, i * NL:(i + 1) * NL, 0:fdim]
                            .rearrange("p c (d h) -> p c d h", h=HEADS),
                            in1=EX[:, 0:NL].unsqueeze(2)
                            .broadcast_to([128, NL, D, HEADS]),
                            op=mybir.AluOpType.mult)
                        nc.vector.tensor_tensor(
                            out=R[:, NL:CH, 0:fdim]
                            .rearrange("p c (d h) -> p c d h", h=HEADS),
                            in0=ghi[:, i * NH:(i + 1) * NH, 0:fdim]
                            .rearrange("p c (d h) -> p c d h", h=HEADS),
                            in1=EX[:, NL:CH].unsqueeze(2)
                            .broadcast_to([128, NH, D, HEADS]),
                            op=mybir.AluOpType.mult)
                        if layer == 0:
                            # fused denominator columns
                            nc.vector.tensor_copy(
                                R[:, :, fdim:fdim + 8], EX[:])
                            P1 = pp2.tile([128, 136], F32, tag="P1")
                            for ch in range(CH):
                                nc.tensor.matmul(
                                    P1[:], inc[:, :, ch], R[:, ch, :],
                                    start=(ch == 0), stop=(ch == CH - 1))
                            post_fn(t, P1, None, op, pp2)
                        else:
                            P1 = pp2.tile([128, 512], F32, tag="P1")
                            for ch in range(CH):
                                nc.tensor.matmul(
                                    P1[:], inc[:, :, ch], R[:, ch, :],
                                    start=(ch == 0), stop=(ch == CH - 1))
                            P2 = pp2.tile([128, 8], F32, tag="P2")
                            for ch in range(CH):
                                nc.tensor.matmul(
                                    P2[:], inc[:, :, ch], EX[:, ch, :],
                                    start=(ch == 0), stop=(ch == CH - 1))
                            post_fn(t, P1, P2, op, pp2)
                        if agin_hook is not None:
                            agin_hook(t)

        # ---- L0 post: softmax-div, +b0, ELU, transpose, f8, store --------
        def post0(t, P1, _, op, pp2):
            rows = 128 if t < LT - 1 else LAST_ROWS
            k = min(int(t) // 12, 3)
            col0 = (t - AGB[k]) * 128
            r8 = op.tile([128, 8], F32, tag="r8")
            nc.vector.tensor_scalar_add(r8[:], P1[:, 128:136], 1e-16)
            nc.vector.reciprocal(r8[:], r8[:])
            z = op.tile([128, NHID], F32, tag="z")
            nc.vector.tensor_tensor(
                out=z[:].rearrange("p (d h) -> p d h", h=HEADS),
                in0=P1[:, 0:128].rearrange("p (d h) -> p d h", h=HEADS),
                in1=r8[:].unsqueeze(1).broadcast_to([128, 16, HEADS]),
                op=mybir.AluOpType.mult)
            nc.vector.tensor_tensor(out=z[:], in0=z[:], in1=b0s[:],
                                    op=mybir.AluOpType.add)
            zm = op.tile([128, NHID], F32, tag="zm")
            nc.vector.tensor_scalar_min(zm[:], z[:], 0.0)
            nc.scalar.activation(zm[:], zm[:],
                                 mybir.ActivationFunctionType.Exp)
            zp = op.tile([128, NHID], F32, tag="zp")
            nc.vector.tensor_scalar_max(zp[:], z[:], 0.0)
            nc.vector.tensor_tensor(out=zp[:], in0=zp[:], in1=zm[:],
                                    op=mybir.AluOpType.add)
            h1 = op.tile([128, NHID], F16, tag="h1")
            nc.vector.tensor_scalar_add(h1[:], zp[:], -1.0)
            pst = pp2.tile([128, 128], F16, tag="pst")
            nc.tensor.transpose(pst[:], h1[:], idents[:])
            h8 = op.tile([128, 128], I8, tag="h8")
            nc.vector.tensor_copy(h8[:].bitcast(F8), pst[:])
            nc.sync.dma_start(
                out=agin[k][:, col0:col0 + rows], in_=h8[:, 0:rows])

        # AllGather chunks fire as soon as their agin columns are complete
        def agin_hook(t):
            for k in range(4):
                if t == AGB[k + 1] - 1:
                    nc.gpsimd.collective_compute(
                        "AllGather", mybir.AluOpType.bypass,
                        replica_groups=[list(range(NCORES))],
                        ins=[agin[k][:]], outs=[agout[k][:]])

        edge_phase(0, STB, t0lo, t0hi, t0ad, NHID, post0, agin_hook)

        pid = nc.partition_id(engines=[mybir.EngineType.SP])
        sregs = [nc.sync.snap(((j + pid) % NCORES) * 128)
                 for j in range(NCORES)]

        # ---------------- phase D: layer-1 tables -------------------------
        ngt = (N + 127) // 128   # 391
        sts = []
        for si in range((ngt + SD - 1) // SD):
            r0 = si * SD * 128
            r1 = min(r0 + SD * 128, ngt * 128)
            r1c = min(r1, N)
            dep = -1
            r = r0
            while r < r1c:
                j = r // NLOC
                cl = r - j * NLOC
                k = next(kk for kk in range(4) if cl < CHB[kk + 1])
                end = min(r1c, j * NLOC + CHB[k + 1])
                if j > 0:
                    dep = max(dep, k)
                r = end
            sts.append((dep, si, r0, r1, r1c))
        sts.sort()

        with ExitStack() as pd:
            xp1 = pd.enter_context(tc.tile_pool(name="pd_x", bufs=3))
            pp1 = pd.enter_context(
                tc.tile_pool(name="pd_ps", bufs=2, space="PSUM"))
            rp1 = pd.enter_context(tc.tile_pool(name="pd_row", bufs=3))
            for _, si, r0, r1, r1c in sts:
                nt = (r1 - r0) // 128
                hx = xp1.tile([128, SD * 128], I8, tag="hx")
                r = r0
                while r < r1c:
                    j = r // NLOC
                    cl = r - j * NLOC
                    k = next(kk for kk in range(4) if cl < CHB[kk + 1])
                    end = min(r1c, j * NLOC + CHB[k + 1])
                    seg = end - r
                    if j == 0:
                        nc.sync.dma_start(
                            out=hx[:, r - r0:r - r0 + seg],
                            in_=agin[k][:, cl - CHB[k]:cl - CHB[k] + seg])
                    else:
                        nc.sync.dma_start(
                            out=hx[:, r - r0:r - r0 + seg],
                            in_=agout[k][bass.ds(sregs[j], 128),
                                         cl - CHB[k]:cl - CHB[k] + seg])
                    r = end
                if r1c < r1:
                    nc.vector.memset(hx[:, r1c - r0:r1 - r0], 0)
                row = rp1.tile([128, SD, 520], F16, tag="row")
                for g2 in range(nt):
                    ps = pp1.tile([128, 512], F32, tag=f"ps{g2 % 2}")
                    nc.tensor.matmul(ps[:],
                                     hx[:, g2 * 128:(g2 + 1) * 128]
                                     .bitcast(F8),
                                     W1s[:], start=True, stop=True)
                    psa = pp1.tile([128, 16], F32, tag=f"psa{g2 % 2}")
                    nc.tensor.matmul(psa[:],
                                     hx[:, g2 * 128:(g2 + 1) * 128]
                                     .bitcast(F8),
                                     W1as[:], start=True, stop=True)
                    if g2 % 2:
                        nc.scalar.copy(row[:, g2, 0:512], ps[:])
                    else:
                        nc.vector.tensor_copy(row[:, g2, 0:512], ps[:])
                    nc.vector.tensor_copy(row[:, g2, 512:520],
                                          psa[:, 0:8])
                    g = si * SD + g2
                    if g < LT:
                        nc.vector.tensor_copy(
                            adbal2[:, g * 8:(g + 1) * 8], psa[:, 8:16])
                weng = nc.scalar if si % 2 else nc.sync
                if r1 <= SPLIT:
                    weng.dma_start(
                        out=t1lo[r0:r1, 0:520]
                        .rearrange("(g p) w -> p g w", p=128),
                        in_=row[:, 0:nt, :])
                elif r0 >= SPLIT:
                    weng.dma_start(
                        out=t1hi[r0 - SPLIT:r1 - SPLIT, 0:520]
                        .rearrange("(g p) w -> p g w", p=128),
                        in_=row[:, 0:nt, :])
                else:
                    nlo = (SPLIT - r0) // 128
                    weng.dma_start(
                        out=t1lo[r0:SPLIT, 0:520]
                        .rearrange("(g p) w -> p g w", p=128),
                        in_=row[:, 0:nlo, :])
                    weng.dma_start(
                        out=t1hi[0:r1 - SPLIT, 0:520]
                        .rearrange("(g p) w -> p g w", p=128),
                        in_=row[:, nlo:nt, :])
            nc.sync.dma_start(
                out=t1ad[:, 0:8].rearrange("(t p) w -> p t w", p=128),
                in_=adbal2[:].rearrange("p (t w) -> p t w", w=8))

        # ---------------- phase E: layer-1 edges + epilogue ---------------
        fpool = stk.enter_context(tc.tile_pool(name="fin", bufs=1))
        zbig = fpool.tile([128, LT * NCLASS], F32)
        nmxb = fpool.tile([128, LT], F32)
        seb = fpool.tile([128, LT], F32)

        def post1(t, P1, P2, op, pp2):
            r8 = op.tile([128, 8], F32, tag="r8")
            nc.vector.tensor_scalar_add(r8[:], P2[:], 1e-16)
            nc.vector.reciprocal(r8[:], r8[:])
            nc.vector.tensor_scalar_mul(r8[:], r8[:], 1.0 / HEADS)
            zw = op.tile([128, 512], F32, tag="zw")
            nc.vector.tensor_tensor(
                out=zw[:].rearrange("p (d h) -> p d h", h=HEADS),
                in0=P1[:].rearrange("p (d h) -> p d h", h=HEADS),
                in1=r8[:].unsqueeze(1).broadcast_to([128, 64, HEADS]),
                op=mybir.AluOpType.mult)
            z = zbig[:, t * NCLASS:(t + 1) * NCLASS]
            nc.vector.reduce_sum(
                z, zw[:].rearrange("p (d h) -> p d h", h=HEADS),
                axis=mybir.AxisListType.X)
            nc.vector.tensor_tensor(out=z, in0=z, in1=b1s[:],
                                    op=mybir.AluOpType.add)
            nmx = nmxb[:, t:t + 1]
            nc.vector.reduce_max(nmx, z, axis=mybir.AxisListType.X,
                                 negate=True)
            ez = op.tile([128, NCLASS], F32, tag="ez")
            nc.scalar.activation(ez[:], z,
                                 mybir.ActivationFunctionType.Exp,
                                 bias=nmx, accum_out=seb[:, t:t + 1])

        edge_phase(1, STE, t1lo, t1hi, t1ad, 512, post1, None)

        # batched log-softmax tail
        nc.scalar.activation(seb[:], seb[:],
                             mybir.ActivationFunctionType.Ln)
        nc.vector.tensor_tensor(
            out=zbig[:].rearrange("p (t c) -> p t c", c=NCLASS),
            in0=zbig[:].rearrange("p (t c) -> p t c", c=NCLASS),
            in1=nmxb[:].unsqueeze(-1).broadcast_to([128, LT, NCLASS]),
            op=mybir.AluOpType.add)
        nc.vector.tensor_tensor(
            out=zbig[:].rearrange("p (t c) -> p t c", c=NCLASS),
            in0=zbig[:].rearrange("p (t c) -> p t c", c=NCLASS),
            in1=seb[:].unsqueeze(-1).broadcast_to([128, LT, NCLASS]),
            op=mybir.AluOpType.subtract)
        nfull = (LT - 1) * 128
        nc.sync.dma_start(
            out=out[0:nfull, :].rearrange("(t p) c -> p t c", p=128),
            in_=zbig[:].rearrange("p (t c) -> p t c", c=NCLASS)
            [:, 0:LT - 1, :])
        nc.sync.dma_start(
            out=out[nfull:nfull + LAST_ROWS, :],
            in_=zbig[0:LAST_ROWS, (LT - 1) * NCLASS:LT * NCLASS])

    nc.compile()
    return nc


# --------------------------------------------------------------------------
# entry point
# --------------------------------------------------------------------------

def kernel(**inputs) -> np.ndarray:
    NLk, NHk, in_maps = _prep_inputs(**inputs)
    key = (NLk, NHk)
    if key not in _cache:
        _cache[key] = build(NLk, NHk)
    nc = _cache[key]
    res = run_bass_kernel_spmd(nc, in_maps, list(range(NCORES)))
    return np.concatenate([res.results[c]["out"] for c in range(NCORES)], 0)
